# revision 1
# baseline (speedup 1.0000x reference)
"""GCN message-passing kernel for Trainium2 (8 NeuronCores, Bass/Tile).

Math (reference): h = x @ W; msg = h[src] * w_e; agg = segment_sum(msg, dst);
y = BETA*z + (C-BETA)*relu(z) with z = agg + b.

We use linearity to aggregate x first: agg_x = segment_sum(x[src] * w_e, dst),
then y = f(agg_x @ W + b). Per-core dst-sharding (12500 nodes each); edges are
routed to the dst-owner core on the host.

Device algorithm per core:
  - Edges sorted by (dst_block, src_quartile), padded per (block, quartile)
    to multiples of 128 (dummy edges have w=0), with a SHARED (across cores)
    static chunk schedule K[block][quartile] = max over cores.
  - dma_gather pulls x[src] rows (512B) into SBUF tiles M [128 edge, 128 feat]
    (gpsimd custom op; int16 indices limited to 32767 -> 4 src-range tables).
  - One-hot matmul per 128-edge chunk: ST[e, d] = (iota_d == dst_e) * w_e
    built in ONE vector op (tensor_scalar is_equal+mult with per-partition
    operands); PSUM accumulates aggT[feat, dst_block] += M.T @ ST.
  - Per block: aggT -> SBUF, z.T = W.T @ aggT (+bias), y = BETA*z +
    (C-BETA)*relu(z) via one ACT + two DVE ops, DMA to the transposed
    output yT [128, 12544]. Host transposes back and trims.
"""
import numpy as np
from contextlib import ExitStack

N_NODES = 100000
N_EDGES = 1600000
D = 128
P = 8
NDST = N_NODES // P            # 12500
NBLK = (NDST + 127) // 128     # 98
NCLS = 4
CLSZ = N_NODES // NCLS         # 25000
SEGBLK = 5
BETA = 0.5
C_CONST = 1.0


def _plan_and_pack(src, dst, w):
    """Host-side routing. Returns the static schedule and per-core arrays."""
    src = np.ascontiguousarray(src.astype(np.int64))
    dst = np.ascontiguousarray(dst.astype(np.int64))
    w = np.ascontiguousarray(w.astype(np.float32))

    core = dst // NDST
    dstl = dst - core * NDST
    blk = dstl >> 7
    d128 = (dstl & 127).astype(np.float32)
    cls = src // CLSZ
    idxl = (src - cls * CLSZ).astype(np.int16)

    key = ((core * NBLK + blk) * NCLS + cls).astype(np.int64)
    order = np.argsort(key, kind="stable")
    key_s = key[order]

    cnt = np.bincount(key, minlength=P * NBLK * NCLS).reshape(P, NBLK, NCLS)
    Kbq = (cnt + 127) // 128
    Kbq = Kbq.max(axis=0)                      # [NBLK, NCLS] shared schedule
    Kbq[:, 0] = np.maximum(Kbq[:, 0], 1)       # ensure start=True exists
    Kb = Kbq.sum(axis=1)                       # chunks per block
    C = int(Kb.sum())                          # total chunks per core

    # segments of SEGBLK blocks; within a segment, chunk columns are laid out
    # class-major: [q=0: all blocks' chunks][q=1: ...]... (gather-call order)
    seg_of_blk = np.arange(NBLK) // SEGBLK
    nseg = int(seg_of_blk.max()) + 1
    col_of = np.zeros((NBLK, NCLS), np.int64)  # first chunk col of (b, q)
    seg_base = np.zeros(nseg + 1, np.int64)    # first col of each segment
    segs = []                                  # per segment: (b0, b1, [(q, colbase, nchunks)])
    colp = 0
    for s in range(nseg):
        b0, b1 = s * SEGBLK, min((s + 1) * SEGBLK, NBLK)
        seg_base[s] = colp
        calls = []
        for q in range(NCLS):
            callbase = colp
            for b in range(b0, b1):
                col_of[b, q] = colp
                colp += int(Kbq[b, q])
            if colp > callbase:
                calls.append((q, callbase, colp - callbase))
        segs.append((b0, b1, calls))
    seg_base[nseg] = colp
    assert colp == C

    # group start of each (core, blk, cls) in the sorted edge list
    cntf = cnt.reshape(-1)
    gstart = np.zeros(P * NBLK * NCLS, np.int64)
    np.cumsum(cntf[:-1], out=gstart[1:])
    grank = np.arange(N_EDGES, dtype=np.int64) - gstart[key_s]

    co = core[order]
    bo = blk[order]
    qo = cls[order]
    chunkpos = col_of[bo, qo] + (grank >> 7)   # global chunk column
    lane = (grank & 127).astype(np.int64)

    dsel = np.zeros((P, 128, C), np.float32)
    wts = np.zeros((P, 128, C), np.float32)
    idxw16 = np.zeros((P, 16, C * 8), np.int16)

    dsel[co, lane, chunkpos] = d128[order]
    wts[co, lane, chunkpos] = w[order]
    idxw16[co, lane & 15, chunkpos * 8 + (lane >> 4)] = idxl[order]

    idxw = np.tile(idxw16, (1, 8, 1))          # replicate to 128 partitions

    meta = {
        "C": C,
        "Kbq": Kbq,
        "col_of": col_of,
        "seg_base": seg_base,
        "segs": segs,
        "kseg_max": int((seg_base[1:] - seg_base[:-1]).max()),
    }
    return meta, dsel, wts, idxw


def _build_nc(meta):
    import concourse.bacc as bacc
    import concourse.tile as tile
    import concourse.mybir as mybir
    from drainfix_embedded import fix_excess_waits

    C = meta["C"]
    Kbq = meta["Kbq"]
    col_of = meta["col_of"]
    seg_base = meta["seg_base"]
    segs = meta["segs"]
    kseg_max = meta["kseg_max"]
    f32 = mybir.dt.float32

    nc = bacc.Bacc("TRN2", target_bir_lowering=False, debug=False, num_devices=P)
    xtab = nc.declare_dram_parameter("xtab", [N_NODES, D], f32, isOutput=False)
    dsel_d = nc.declare_dram_parameter("dsel", [128, C], f32, isOutput=False)
    wts_d = nc.declare_dram_parameter("wts", [128, C], f32, isOutput=False)
    idxw_d = nc.declare_dram_parameter("idxw", [128, C * 8], mybir.dt.int16, isOutput=False)
    wmat_d = nc.declare_dram_parameter("wmat", [D, D], f32, isOutput=False)
    bb_d = nc.declare_dram_parameter("bb", [D, 1], f32, isOutput=False)   # BETA*b
    cb_d = nc.declare_dram_parameter("cb", [D, 1], f32, isOutput=False)   # (C-BETA)*b
    iota_d = nc.declare_dram_parameter("iotaf", [128, 128], f32, isOutput=False)
    yt = nc.declare_dram_parameter("yt", [128, NBLK * 128], f32, isOutput=True)

    with tile.TileContext(nc) as tc:
        with ExitStack() as ctx:
            consts = ctx.enter_context(tc.tile_pool(name="consts", bufs=1))
            metap = ctx.enter_context(tc.tile_pool(name="meta", bufs=1))
            gpool = ctx.enter_context(tc.tile_pool(name="gseg", bufs=2))
            stp = ctx.enter_context(tc.tile_pool(name="st", bufs=6))
            evp = ctx.enter_context(tc.tile_pool(name="ev", bufs=3))
            yp = ctx.enter_context(tc.tile_pool(name="y", bufs=3))
            pagg = ctx.enter_context(tc.tile_pool(name="pagg", bufs=4, space="PSUM"))
            pz = ctx.enter_context(tc.tile_pool(name="pz", bufs=2, space="PSUM"))

            iota_sb = consts.tile([128, 128], f32)
            nc.sync.dma_start(iota_sb[:], iota_d[:])
            w_sb = consts.tile([128, 128], f32)
            nc.sync.dma_start(w_sb[:], wmat_d[:])
            bb_sb = consts.tile([128, 1], f32)
            nc.sync.dma_start(bb_sb[:], bb_d[:])
            cb_sb = consts.tile([128, 1], f32)
            nc.sync.dma_start(cb_sb[:], cb_d[:])

            dsel_sb = metap.tile([128, C], f32)
            nc.sync.dma_start(dsel_sb[:], dsel_d[:])
            wts_sb = metap.tile([128, C], f32)
            nc.sync.dma_start(wts_sb[:], wts_d[:])
            idxw_sb = metap.tile([128, C * 8], mybir.dt.int16)
            nc.sync.dma_start(idxw_sb[:], idxw_d[:])

            for (b0, b1, calls) in segs:
                s0 = int(seg_base[b1 // SEGBLK if b1 % SEGBLK else b1 // SEGBLK - 1])
                segc0 = int(col_of[b0, 0])           # first col of this segment
                kseg = int(sum(n for (_, _, n) in calls))
                g = gpool.tile([128, kseg_max * 128], f32, tag="gseg")
                g3 = g[:].rearrange("p (k d) -> p k d", d=D)
                for (q, callbase, nch) in calls:
                    lc0 = callbase - segc0
                    nidx = nch * 128
                    nc.gpsimd.dma_gather(
                        g3[:, lc0 : lc0 + nch, :],
                        xtab[q * CLSZ : (q + 1) * CLSZ, :],
                        idxw_sb[:, callbase * 8 : (callbase + nch) * 8],
                        nidx,
                        nidx,
                        D,
                        single_packet=False,
                    )
                for b in range(b0, b1):
                    pa = pagg.tile([128, 128], f32, tag="pagg")
                    nchunks_b = int(Kbq[b].sum())
                    done = 0
                    for q in range(NCLS):
                        for i in range(int(Kbq[b, q])):
                            cg = int(col_of[b, q]) + i   # global col
                            cl = cg - segc0              # col in segment tile
                            st = stp.tile([128, 128], f32, tag="st")
                            nc.vector.tensor_scalar(
                                st[:],
                                iota_sb[:],
                                dsel_sb[:, cg : cg + 1],
                                wts_sb[:, cg : cg + 1],
                                op0=mybir.AluOpType.is_equal,
                                op1=mybir.AluOpType.mult,
                            )
                            nc.tensor.matmul(
                                out=pa[:],
                                lhsT=g3[:, cl, :],
                                rhs=st[:],
                                start=(done == 0),
                                stop=(done == nchunks_b - 1),
                            )
                            done += 1
                    aggT = evp.tile([128, 128], f32, tag="ev")
                    nc.scalar.copy(aggT[:], pa[:])
                    z = pz.tile([128, 128], f32, tag="pz")
                    nc.tensor.matmul(
                        out=z[:], lhsT=w_sb[:], rhs=aggT[:], start=True, stop=True
                    )
                    t1 = yp.tile([128, 128], f32, tag="t1")
                    nc.scalar.activation(
                        t1[:],
                        z[:],
                        mybir.ActivationFunctionType.Relu,
                        bias=cb_sb[:],
                        scale=(C_CONST - BETA),
                    )
                    t2 = yp.tile([128, 128], f32, tag="t2")
                    nc.vector.tensor_scalar(
                        t2[:],
                        z[:],
                        BETA,
                        bb_sb[:],
                        op0=mybir.AluOpType.mult,
                        op1=mybir.AluOpType.add,
                    )
                    yb = yp.tile([128, 128], f32, tag="yb")
                    nc.vector.tensor_tensor(
                        yb[:], t1[:], t2[:], op=mybir.AluOpType.add
                    )
                    nc.sync.dma_start(yt[:, b * 128 : (b + 1) * 128], yb[:])

    nc.compile()
    fix_excess_waits(nc)
    return nc


def kernel(x, edge_index, edge_weight, W, b):
    x = np.asarray(x, np.float32)
    edge_index = np.asarray(edge_index)
    edge_weight = np.asarray(edge_weight, np.float32)
    W = np.asarray(W, np.float32)
    b = np.asarray(b, np.float32)

    meta, dsel, wts, idxw = _plan_and_pack(edge_index[0], edge_index[1], edge_weight)
    nc = _build_nc(meta)

    from concourse.bass_utils import run_bass_kernel_spmd

    iota = np.tile(np.arange(128, dtype=np.float32), (128, 1))
    bb = (BETA * b).reshape(D, 1).astype(np.float32)
    cb = ((C_CONST - BETA) * b).reshape(D, 1).astype(np.float32)
    in_maps = []
    for c in range(P):
        in_maps.append(
            {
                "xtab": x,
                "dsel": dsel[c],
                "wts": wts[c],
                "idxw": idxw[c],
                "wmat": W,
                "bb": bb,
                "cb": cb,
                "iotaf": iota,
            }
        )
    res = run_bass_kernel_spmd(nc, in_maps, list(range(P)))
    y = np.empty((N_NODES, D), np.float32)
    for c in range(P):
        y[c * NDST : (c + 1) * NDST] = res.results[c]["yt"][:, :NDST].T
    return y


# ---------------------------------------------------------------------------
# Embedded walrus workaround (kernel.py must be self-contained): split excess
# sem waits onto preceding NoOps — this walrus build rejects >1 sync wait on
# Drain and on the extended DMA instructions.
import sys as _sys
import types as _types

_dfx_src = '''
import concourse.mybir as mybir

LIMIT_DEFAULT = 1
LIMIT_BY_TYPE = {mybir.InstDrain: 1}


def fix_excess_waits(nc):
    fixed = 0

    def limit_for(ins):
        for t, lim in LIMIT_BY_TYPE.items():
            if isinstance(ins, t):
                return lim
        return LIMIT_DEFAULT

    def walk(block):
        nonlocal fixed
        insts = block.instructions
        i = 0
        while i < len(insts):
            ins = insts[i]
            si = getattr(ins, "sync_info", None)
            lim = limit_for(ins)
            if si is not None and len(si.on_wait) > lim:
                waits = list(si.on_wait)
                excess, keep = waits[:-lim], waits[-lim:]
                pos = i
                for j in range(0, len(excess), LIMIT_DEFAULT):
                    nop = mybir.InstNoOp(name=f"{ins.name}_xw{j}", ins=[], outs=[])
                    nop.engine = ins.engine
                    nop.sync_info = mybir.SyncInfo(
                        on_wait=excess[j : j + LIMIT_DEFAULT], on_update=[]
                    )
                    try:
                        nc.register_instruction(nop)
                    except Exception:
                        pass
                    insts.insert(pos, nop)
                    pos += 1
                    i += 1
                si.on_wait = keep
                fixed += 1
            i += 1
        for sub in getattr(block, "blocks", []) or []:
            walk(sub)

    for fn in nc.m.functions:
        for b in fn.blocks:
            walk(b)
    return fixed
'''

_mod = _types.ModuleType("drainfix_embedded")
exec(_dfx_src, _mod.__dict__)
_sys.modules["drainfix_embedded"] = _mod


# revision 16
# speedup vs baseline: 99.4167x; 99.4167x over previous
"""GCN message-passing kernel for Trainium2 (8 NeuronCores, Bass/Tile).

Math (reference): h = x @ W; msg = h[src] * w_e; agg = segment_sum(msg, dst);
y = BETA*z + (C-BETA)*relu(z) with z = agg + b.

We use linearity to aggregate x first: agg_x = segment_sum(x[src] * w_e, dst),
then y = f(agg_x @ W + b). Per-core dst-sharding (12500 nodes each); edges are
routed to the dst-owner core on the host.

Device algorithm per core:
  - Edges sorted by (dst_block, src_quartile), padded per (block, quartile)
    to multiples of 128 (dummy edges have w=0), with a SHARED (across cores)
    static chunk schedule K[block][quartile] = max over cores.
  - dma_gather pulls x[src] rows (512B) into SBUF tiles M [128 edge, 128 feat]
    (gpsimd custom op; int16 indices limited to 32767 -> 4 src-range tables).
  - One-hot matmul per 128-edge chunk: ST[e, d] = (iota_d == dst_e) * w_e
    built in ONE vector op (tensor_scalar is_equal+mult with per-partition
    operands); PSUM accumulates aggT[feat, dst_block] += M.T @ ST.
  - Per block: aggT -> SBUF, z.T = W.T @ aggT (+bias), y = BETA*z +
    (C-BETA)*relu(z) via one ACT + two DVE ops, DMA to the transposed
    output yT [128, 12544]. Host transposes back and trims.
"""
import numpy as np
from contextlib import ExitStack

N_NODES = 100000
N_EDGES = 1600000
D = 128
P = 8
NDST = N_NODES // P            # 12500
NBLK = (NDST + 127) // 128     # 98
NCLS = 4
CLSZ = N_NODES // NCLS         # 25000
SEGBLK = 8
BETA = 0.5
C_CONST = 1.0


def _plan_and_pack(src, dst, w):
    """Host-side routing. Returns the static schedule and per-core arrays."""
    src = np.ascontiguousarray(src.astype(np.int64))
    dst = np.ascontiguousarray(dst.astype(np.int64))
    w = np.ascontiguousarray(w.astype(np.float32))

    core = dst // NDST
    dstl = dst - core * NDST
    blk = dstl >> 7
    d128 = (dstl & 127).astype(np.float32)
    cls = src // CLSZ
    idxl = (src - cls * CLSZ).astype(np.int16)

    key = ((core * NBLK + blk) * NCLS + cls).astype(np.int64)
    order = np.argsort(key, kind="stable")
    key_s = key[order]

    cnt = np.bincount(key, minlength=P * NBLK * NCLS).reshape(P, NBLK, NCLS)
    Kbq = (cnt + 127) // 128
    Kbq = Kbq.max(axis=0)                      # [NBLK, NCLS] shared schedule
    Kbq[:, 0] = np.maximum(Kbq[:, 0], 1)       # ensure start=True exists
    Kb = Kbq.sum(axis=1)                       # chunks per block
    C = int(Kb.sum())                          # total chunks per core

    # segments of SEGBLK blocks; within a segment, chunk columns are laid out
    # class-major: [q=0: all blocks' chunks][q=1: ...]... (gather-call order)
    seg_of_blk = np.arange(NBLK) // SEGBLK
    nseg = int(seg_of_blk.max()) + 1
    col_of = np.zeros((NBLK, NCLS), np.int64)  # first chunk col of (b, q)
    seg_base = np.zeros(nseg + 1, np.int64)    # first col of each segment
    segs = []                                  # per segment: (b0, b1, [(q, colbase, nchunks)])
    colp = 0
    for s in range(nseg):
        b0, b1 = s * SEGBLK, min((s + 1) * SEGBLK, NBLK)
        seg_base[s] = colp
        calls = []
        for q in range(NCLS):
            callbase = colp
            for b in range(b0, b1):
                col_of[b, q] = colp
                colp += int(Kbq[b, q])
            if colp > callbase:
                calls.append((q, callbase, colp - callbase))
        segs.append((b0, b1, calls))
    seg_base[nseg] = colp
    assert colp == C

    # group start of each (core, blk, cls) in the sorted edge list
    cntf = cnt.reshape(-1)
    gstart = np.zeros(P * NBLK * NCLS, np.int64)
    np.cumsum(cntf[:-1], out=gstart[1:])
    grank = np.arange(N_EDGES, dtype=np.int64) - gstart[key_s]

    co = core[order]
    bo = blk[order]
    qo = cls[order]
    chunkpos = col_of[bo, qo] + (grank >> 7)   # global chunk column
    lane = (grank & 127).astype(np.int64)

    dsel = np.zeros((P, 128, C), np.float32)
    wts = np.zeros((P, 128, C), np.float32)
    idxw16 = np.zeros((P, 16, C * 8), np.int16)

    dsel[co, lane, chunkpos] = d128[order]
    wts[co, lane, chunkpos] = w[order]
    idxw16[co, lane & 15, chunkpos * 8 + (lane >> 4)] = idxl[order]

    idxw = np.tile(idxw16, (1, 8, 1))          # replicate to 128 partitions

    meta = {
        "C": C,
        "Kbq": Kbq,
        "col_of": col_of,
        "seg_base": seg_base,
        "segs": segs,
        "kseg_max": int((seg_base[1:] - seg_base[:-1]).max()),
    }
    return meta, dsel, wts, idxw


def _build_nc(meta, reps=1, skip_gather=False, skip_st=False, skip_mm=False,
              skip_final=False, st_bf16=True, st_any=True):
    import concourse.bacc as bacc
    import concourse.tile as tile
    import concourse.mybir as mybir
    from drainfix_embedded import fix_excess_waits

    C = meta["C"]
    Kbq = meta["Kbq"]
    col_of = meta["col_of"]
    seg_base = meta["seg_base"]
    segs = meta["segs"]
    kseg_max = meta["kseg_max"]
    f32 = mybir.dt.float32

    nc = bacc.Bacc("TRN2", target_bir_lowering=False, debug=False, num_devices=P)
    x_dt = mybir.dt.bfloat16 if st_bf16 else f32
    xtab = nc.declare_dram_parameter("xtab", [N_NODES, D], x_dt, isOutput=False)
    meta_dt = mybir.dt.bfloat16 if st_bf16 else f32
    dsel_d = nc.declare_dram_parameter("dsel", [128, C], f32, isOutput=False)
    wts_d = nc.declare_dram_parameter("wts", [128, C], f32, isOutput=False)
    idxw_d = nc.declare_dram_parameter("idxw", [128, C * 8], mybir.dt.int16, isOutput=False)
    wmat_d = nc.declare_dram_parameter("wmat", [D, D], f32, isOutput=False)
    bb_d = nc.declare_dram_parameter("bb", [D, 1], f32, isOutput=False)   # BETA*b
    cb_d = nc.declare_dram_parameter("cb", [D, 1], f32, isOutput=False)   # (C-BETA)*b
    iota_dt = mybir.dt.bfloat16 if st_bf16 else f32
    iota_d = nc.declare_dram_parameter("iotaf", [128, 128], iota_dt, isOutput=False)
    yt = nc.declare_dram_parameter("yt", [128, NBLK * 128], f32, isOutput=True)

    with tile.TileContext(nc) as tc:
        with ExitStack() as ctx:
            consts = ctx.enter_context(tc.tile_pool(name="consts", bufs=1))
            metap = ctx.enter_context(tc.tile_pool(name="meta", bufs=1))
            gpool = ctx.enter_context(tc.tile_pool(name="gseg", bufs=3))
            stp = ctx.enter_context(tc.tile_pool(name="st", bufs=10))
            evp = ctx.enter_context(tc.tile_pool(name="ev", bufs=3))
            yp = ctx.enter_context(tc.tile_pool(name="y", bufs=3))
            pagg = ctx.enter_context(tc.tile_pool(name="pagg", bufs=6, space="PSUM"))
            pz = ctx.enter_context(tc.tile_pool(name="pz", bufs=2, space="PSUM"))

            iota_sb = consts.tile([128, 128], iota_dt)
            nc.sync.dma_start(iota_sb[:], iota_d[:])
            w_sb = consts.tile([128, 128], f32)
            nc.sync.dma_start(w_sb[:], wmat_d[:])
            bb_sb = consts.tile([128, 1], f32)
            nc.sync.dma_start(bb_sb[:], bb_d[:])
            cb_sb = consts.tile([128, 1], f32)
            nc.sync.dma_start(cb_sb[:], cb_d[:])

            dsel_sb = metap.tile([128, C], f32)
            wts_sb = metap.tile([128, C], f32)
            idxw_sb = metap.tile([128, C * 8], mybir.dt.int16)

            from contextlib import nullcontext
            loop_cm = tc.For_i(0, reps, 1) if reps > 1 else nullcontext()
            with loop_cm:
             for si, (b0, b1, calls) in enumerate(segs):
                c0s = int(seg_base[si])
                c1s = int(seg_base[si + 1])
                nc.sync.dma_start(idxw_sb[:, c0s * 8 : c1s * 8], idxw_d[:, c0s * 8 : c1s * 8])
                nc.sync.dma_start(dsel_sb[:, c0s:c1s], dsel_d[:, c0s:c1s])
                nc.sync.dma_start(wts_sb[:, c0s:c1s], wts_d[:, c0s:c1s])

             for (b0, b1, calls) in segs:
                s0 = int(seg_base[b1 // SEGBLK if b1 % SEGBLK else b1 // SEGBLK - 1])
                segc0 = int(col_of[b0, 0])           # first col of this segment
                kseg = int(sum(n for (_, _, n) in calls))
                g = gpool.tile([128, kseg_max * 128], x_dt, tag="gseg")
                g3 = g[:].rearrange("p (k d) -> p k d", d=D)
                for (q, callbase, nch) in calls:
                    if skip_gather:
                        break
                    lc0 = callbase - segc0
                    nidx = nch * 128
                    nc.gpsimd.dma_gather(
                        g3[:, lc0 : lc0 + nch, :],
                        xtab[q * CLSZ : (q + 1) * CLSZ, :],
                        idxw_sb[:, callbase * 8 : (callbase + nch) * 8],
                        nidx,
                        nidx,
                        D,
                        single_packet=False,
                    )
                for b in range(b0, b1):
                    pa = None if skip_mm else pagg.tile([128, 128], f32, tag="pagg")
                    nchunks_b = int(Kbq[b].sum())
                    done = 0
                    for q in range(NCLS):
                        for i in range(int(Kbq[b, q])):
                            cg = int(col_of[b, q]) + i   # global col
                            cl = cg - segc0              # col in segment tile
                            if skip_st:
                                st = iota_sb
                            else:
                                st_dt = mybir.dt.bfloat16 if st_bf16 else f32
                                st = stp.tile([128, 128], st_dt, tag="st")
                                eng = nc.any if st_any else nc.vector
                                eng.tensor_scalar(
                                    st[:],
                                    iota_sb[:],
                                    dsel_sb[:, cg : cg + 1],
                                    wts_sb[:, cg : cg + 1],
                                    op0=mybir.AluOpType.is_equal,
                                    op1=mybir.AluOpType.mult,
                                )
                            if skip_mm:
                                done += 1
                                continue
                            nc.tensor.matmul(
                                out=pa[:],
                                lhsT=g3[:, cl, :],
                                rhs=st[:],
                                start=(done == 0),
                                stop=(done == nchunks_b - 1),
                            )
                            done += 1
                    if skip_final or skip_mm:
                        continue
                    aggT = evp.tile([128, 128], f32, tag="ev")
                    nc.scalar.copy(aggT[:], pa[:])
                    z = pz.tile([128, 128], f32, tag="pz")
                    nc.tensor.matmul(
                        out=z[:], lhsT=w_sb[:], rhs=aggT[:], start=True, stop=True
                    )
                    t1 = yp.tile([128, 128], f32, tag="t1")
                    nc.scalar.activation(
                        t1[:],
                        z[:],
                        mybir.ActivationFunctionType.Relu,
                        bias=cb_sb[:],
                        scale=(C_CONST - BETA),
                    )
                    t2 = yp.tile([128, 128], f32, tag="t2")
                    nc.vector.tensor_scalar(
                        t2[:],
                        z[:],
                        BETA,
                        bb_sb[:],
                        op0=mybir.AluOpType.mult,
                        op1=mybir.AluOpType.add,
                    )
                    yb = yp.tile([128, 128], f32, tag="yb")
                    nc.vector.tensor_tensor(
                        yb[:], t1[:], t2[:], op=mybir.AluOpType.add
                    )
                    nc.sync.dma_start(yt[:, b * 128 : (b + 1) * 128], yb[:])

    nc.compile()
    fix_excess_waits(nc)
    return nc


def kernel(x, edge_index, edge_weight, W, b):
    x = np.asarray(x, np.float32)
    edge_index = np.asarray(edge_index)
    edge_weight = np.asarray(edge_weight, np.float32)
    W = np.asarray(W, np.float32)
    b = np.asarray(b, np.float32)

    meta, dsel, wts, idxw = _plan_and_pack(edge_index[0], edge_index[1], edge_weight)
    nc = _build_nc(meta)

    from concourse.bass_utils import run_bass_kernel_spmd

    import ml_dtypes as _mld
    iota = np.tile(np.arange(128, dtype=np.float32), (128, 1)).astype(_mld.bfloat16)
    bb = (BETA * b).reshape(D, 1).astype(np.float32)
    cb = ((C_CONST - BETA) * b).reshape(D, 1).astype(np.float32)
    import ml_dtypes
    x_cast = x.astype(ml_dtypes.bfloat16)
    in_maps = []
    for c in range(P):
        in_maps.append(
            {
                "xtab": x_cast,
                "dsel": dsel[c],
                "wts": wts[c],
                "idxw": idxw[c],
                "wmat": W,
                "bb": bb,
                "cb": cb,
                "iotaf": iota,
            }
        )
    res = run_bass_kernel_spmd(nc, in_maps, list(range(P)))
    y = np.empty((N_NODES, D), np.float32)
    for c in range(P):
        y[c * NDST : (c + 1) * NDST] = res.results[c]["yt"][:, :NDST].T
    return y


# ---------------------------------------------------------------------------
# Embedded walrus workaround (kernel.py must be self-contained): split excess
# sem waits onto preceding NoOps — this walrus build rejects >1 sync wait on
# Drain and on the extended DMA instructions.
import sys as _sys
import types as _types

_dfx_src = '''
import concourse.mybir as mybir

LIMIT_DEFAULT = 1
LIMIT_BY_TYPE = {mybir.InstDrain: 1}


def fix_excess_waits(nc):
    fixed = 0

    def limit_for(ins):
        for t, lim in LIMIT_BY_TYPE.items():
            if isinstance(ins, t):
                return lim
        return LIMIT_DEFAULT

    def walk(block):
        nonlocal fixed
        insts = block.instructions
        i = 0
        while i < len(insts):
            ins = insts[i]
            si = getattr(ins, "sync_info", None)
            lim = limit_for(ins)
            if si is not None and len(si.on_wait) > lim:
                waits = list(si.on_wait)
                excess, keep = waits[:-lim], waits[-lim:]
                pos = i
                for j in range(0, len(excess), LIMIT_DEFAULT):
                    nop = mybir.InstNoOp(name=f"{ins.name}_xw{j}", ins=[], outs=[])
                    nop.engine = ins.engine
                    nop.sync_info = mybir.SyncInfo(
                        on_wait=excess[j : j + LIMIT_DEFAULT], on_update=[]
                    )
                    try:
                        nc.register_instruction(nop)
                    except Exception:
                        pass
                    insts.insert(pos, nop)
                    pos += 1
                    i += 1
                si.on_wait = keep
                fixed += 1
            i += 1
        for sub in getattr(block, "blocks", []) or []:
            walk(sub)

    for fn in nc.m.functions:
        for b in fn.blocks:
            walk(b)
    return fixed
'''

_mod = _types.ModuleType("drainfix_embedded")
exec(_dfx_src, _mod.__dict__)
_sys.modules["drainfix_embedded"] = _mod
